# revision 5
# baseline (speedup 1.0000x reference)
"""Causal multi-head attention (B=4, T=2048, H=16, hs=64, D=1024) on 8
Trainium2 NeuronCores.

Sharding: tensor-parallel over heads — each core computes 2 heads'
Q/K/V projections + attention, then a partial output projection
(y_partial = O_2h @ Wo[:, core_cols].T).  Host sums the 8 partials and
adds the bias (cheap: one fp32 reduction over 8 arrays).

On-core algorithm (per batch b, per head h):
  xT[b] [D,T] resident in SBUF (8 chunks of [128,T]).
  QT/KT/VT computed 2-head-packed: [128, T] = Wp.T @ xT  (PE, fp32r).
  V transposed per 128-key chunk via PE-transpose into Vtilde [128, 65]
  (65th column = ones, so the attention-times-V matmul also produces the
  softmax denominators).
  Scores are computed transposed, S_T [k=128, q=512] = KT_chunk.T @ QT_blk,
  exp'd on the scalar engine (scale=1/8 folded in, no max subtraction —
  scores are O(1)), causal-masked by elementwise multiply with one of 4
  precomputed 0/1 masks (only the 4 diagonal chunks need it; strictly-upper
  chunks are skipped entirely).
  O_T accumulates in PSUM: [65, 512] += Vtilde.T @ P_T over key chunks.
  Normalisation: recip of denominator row, broadcast across partitions via
  a rank-1 matmul with a ones column, multiply into OT_core.
  Output projection: y[b, 128-row chunk, :] = OT_core_chunk.T @ WoT_core.

All matmuls run as float32r (fp32 exponent, 11-bit mantissa) — full PE
rate at moving-dim 512.  Matmul operands are produced either by DMA from
pre-rounded host data or by compute-engine writes to f32r tiles.
"""

from contextlib import ExitStack

import numpy as np

import concourse.bass as bass
import concourse.mybir as mybir
import concourse.tile as tile
from concourse import bacc

F32 = mybir.dt.float32
F32R = mybir.dt.float32r
EXP = mybir.ActivationFunctionType.Exp

# problem shape (hardcoded per harness contract)
B, T, D, H, HS = 4, 2048, 1024, 16, 64
N_CORES = 8
HPC = H // N_CORES          # heads per core = 2
QB = 512                    # query block (matmul moving dim)
KC = 128                    # key chunk (partition dim)
SCALE = HS ** -0.5


def round_fp32r(a: np.ndarray, mant_bits: int = 11) -> np.ndarray:
    """RNE-round fp32 to fp32r (11-bit mantissa kept, fp32 exponent)."""
    u = np.ascontiguousarray(a, dtype=np.float32).view(np.uint32)
    shift = np.uint32(23 - mant_bits)
    bias = ((u >> shift) & np.uint32(1)) + np.uint32((1 << (shift - 1)) - 1)
    u2 = ((u + bias) >> shift) << shift
    return u2.view(np.float32)


def build_nc(b=B, t=T, d=D, hpc=HPC):
    """Build the per-core program. All cores run the same NEFF; per-core
    data (weight slices) comes in through the input tensors."""
    n_dc = d // 128           # D chunks (contraction for projections)
    n_qb = t // QB            # query blocks
    n_kc = t // KC            # key chunks
    n_tc = t // 128           # T chunks (output projection rows)
    mh = 64 * hpc             # packed head width (=128 for hpc=2)

    nc = bacc.Bacc("TRN2", target_bir_lowering=False, debug=False)

    xT = nc.dram_tensor("xT", [b, d, t], F32R, kind="ExternalInput").ap()
    wq = nc.dram_tensor("wq", [d, mh], F32R, kind="ExternalInput").ap()
    wk = nc.dram_tensor("wk", [d, mh], F32R, kind="ExternalInput").ap()
    wv = nc.dram_tensor("wv", [d, mh], F32R, kind="ExternalInput").ap()
    woT = nc.dram_tensor("woT", [mh, d], F32R, kind="ExternalInput").ap()
    masks = nc.dram_tensor("masks", [4, KC, QB], F32, kind="ExternalInput").ap()
    ident = nc.dram_tensor("ident", [128, 64], F32, kind="ExternalInput").ap()
    y = nc.dram_tensor("y", [b, t, d], F32, kind="ExternalOutput").ap()

    with tile.TileContext(nc) as tc, ExitStack() as ctx:
        consts = ctx.enter_context(tc.tile_pool(name="consts", bufs=1))
        xt_pool = ctx.enter_context(tc.tile_pool(name="xt", bufs=n_dc))
        qkv_pool = ctx.enter_context(tc.tile_pool(name="qkv", bufs=2))
        vtil_pool = ctx.enter_context(tc.tile_pool(name="vtil", bufs=2 * hpc))
        p_pool = ctx.enter_context(tc.tile_pool(name="p", bufs=3))
        ot_pool = ctx.enter_context(tc.tile_pool(name="ot", bufs=2))
        ysb_pool = ctx.enter_context(tc.tile_pool(name="ysb", bufs=2))
        small_pool = ctx.enter_context(tc.tile_pool(name="small", bufs=2))

        ps_proj = ctx.enter_context(tc.tile_pool(name="psp", bufs=2, space="PSUM"))
        ps_s = ctx.enter_context(tc.tile_pool(name="pss", bufs=2, space="PSUM"))
        ps_av = ctx.enter_context(tc.tile_pool(name="psav", bufs=2, space="PSUM"))

        # --- constants ---
        wq_sb = consts.tile([128, n_dc, mh], F32R, tag="wq")
        wk_sb = consts.tile([128, n_dc, mh], F32R, tag="wk")
        wv_sb = consts.tile([128, n_dc, mh], F32R, tag="wv")
        for w_sb, w_dram in ((wq_sb, wq), (wk_sb, wk), (wv_sb, wv)):
            nc.sync.dma_start(w_sb[:], w_dram.rearrange("(c p) m -> p c m", p=128))
        woT_sb = consts.tile([mh, d], F32R, tag="wo")
        nc.sync.dma_start(woT_sb[:], woT[:])
        masks_sb = consts.tile([KC, 4, QB], F32, tag="masks")
        nc.sync.dma_start(masks_sb[:], masks.rearrange("d p f -> p d f"))
        ident_sb = consts.tile([128, 64], F32, tag="ident")
        nc.sync.dma_start(ident_sb[:], ident[:])
        # ones column [128,1] for the Vtilde ones-column writes
        ones_f32 = consts.tile([128, 1], F32, tag="ones_f32")
        nc.vector.memset(ones_f32[:], 1.0)

        for bi in range(b):
            # --- load xT chunks ---
            xt = []
            for c in range(n_dc):
                xc = xt_pool.tile([128, t], F32R, tag="xt")
                nc.sync.dma_start(xc[:], xT[bi, c * 128:(c + 1) * 128, :])
                xt.append(xc)

            # --- QKV projections, 2-head packed: out [mh, t] ---
            qt2 = qkv_pool.tile([mh, t], F32R, tag="qt2")
            kt2 = qkv_pool.tile([mh, t], F32R, tag="kt2")
            vt2 = qkv_pool.tile([mh, t], F32, tag="vt2")
            for w_sb, dst in ((wq_sb, qt2), (wk_sb, kt2), (wv_sb, vt2)):
                for nb in range(n_qb):
                    acc = ps_proj.tile([mh, QB], F32, tag="proj")
                    for c in range(n_dc):
                        nc.tensor.matmul(
                            acc[:], w_sb[:, c, :],
                            xt[c][:, nb * QB:(nb + 1) * QB],
                            start=(c == 0), stop=(c == n_dc - 1))
                    nc.vector.tensor_copy(dst[:, nb * QB:(nb + 1) * QB], acc[:])

            # --- V transpose into Vtilde chunks [128, 65] (+ones col) ---
            vtil = []
            for hh in range(hpc):
                vt = vtil_pool.tile([128, n_kc, 65], F32R, tag="vtil")
                for j in range(n_kc):
                    trp = ps_proj.tile([128, 64], F32, tag="proj")
                    nc.tensor.transpose(
                        trp[:], vt2[hh * 64:(hh + 1) * 64, j * KC:(j + 1) * KC],
                        ident_sb[hh * 64:(hh + 1) * 64, :])
                    nc.vector.tensor_copy(vt[:, j, 0:64], trp[:])
                nc.vector.tensor_copy(
                    vt[:, :, 64], ones_f32[:].broadcast_to([128, n_kc]))
                vtil.append(vt)

            # --- attention per head ---
            ot_core = ot_pool.tile([mh, t], F32R, tag="ot")
            for hh in range(hpc):
                qth = qt2[hh * 64:(hh + 1) * 64, :]
                kth = kt2[hh * 64:(hh + 1) * 64, :]
                for qb in range(n_qb):
                    kmax = (qb + 1) * (QB // KC)
                    oacc = ps_av.tile([128, QB], F32, tag="av")
                    for kc2 in range(kmax // 2):
                        # two score chunks share a 2-bank PSUM tile so one
                        # ACT instruction exps both (less per-op overhead)
                        sps = ps_s.tile([KC, 2 * QB], F32, tag="s")
                        for i in range(2):
                            kc = 2 * kc2 + i
                            nc.tensor.matmul(
                                sps[:, i * QB:(i + 1) * QB],
                                kth[:, kc * KC:(kc + 1) * KC],
                                qth[:, qb * QB:(qb + 1) * QB],
                                start=True, stop=True)
                        psb = p_pool.tile([KC, 2 * QB], F32R, tag="p")
                        nc.scalar.activation(psb[:], sps[:], EXP, scale=SCALE)
                        for i in range(2):
                            kc = 2 * kc2 + i
                            dlt = kc * KC - qb * QB
                            if dlt >= 0:
                                nc.gpsimd.tensor_mul(
                                    psb[:, i * QB:(i + 1) * QB],
                                    psb[:, i * QB:(i + 1) * QB],
                                    masks_sb[:, dlt // KC, :])
                            nc.tensor.matmul(
                                oacc[0:65, :], vtil[hh][:, kc, :],
                                psb[:, i * QB:(i + 1) * QB],
                                start=(kc == 0), stop=(kc == kmax - 1))
                    # normalise: recip of denom row, partition-broadcast
                    # (gpsimd), multiply into ot_core
                    recf = small_pool.tile([1, QB], F32, tag="recf")
                    nc.vector.reciprocal(recf[:], oacc[64:65, :])
                    bcs = small_pool.tile([64, QB], F32, tag="bcs")
                    nc.gpsimd.partition_broadcast(bcs[:], recf[:])
                    nc.vector.tensor_mul(
                        ot_core[hh * 64:(hh + 1) * 64, qb * QB:(qb + 1) * QB],
                        oacc[0:64, :], bcs[:])

            # --- output projection: y[bi, tc*128:(tc+1)*128, :] ---
            for tcn in range(n_tc):
                ysb = ysb_pool.tile([128, d], F32, tag="ysb")
                for nb0 in range(0, d, QB):
                    nw = min(QB, d - nb0)
                    op = ps_proj.tile([128, nw], F32, tag="proj")
                    nc.tensor.matmul(
                        op[:], ot_core[:, tcn * 128:(tcn + 1) * 128],
                        woT_sb[:, nb0:nb0 + nw],
                        start=True, stop=True)
                    nc.vector.tensor_copy(ysb[:, nb0:nb0 + nw], op[:])
                nc.sync.dma_start(y[bi, tcn * 128:(tcn + 1) * 128, :], ysb[:])

    nc.compile()
    return nc


_NC_CACHE = {}


def _get_nc():
    if "nc" not in _NC_CACHE:
        _NC_CACHE["nc"] = build_nc()
    return _NC_CACHE["nc"]


def make_masks() -> np.ndarray:
    m = np.zeros((4, KC, QB), np.float32)
    p = np.arange(KC)[:, None]
    f = np.arange(QB)[None, :]
    for i, dlt in enumerate((0, 128, 256, 384)):
        m[i] = (p + dlt <= f).astype(np.float32)
    return m


def make_in_maps(x, Wq, Wk, Wv, Wo):
    xTr = round_fp32r(np.ascontiguousarray(x.transpose(0, 2, 1)))
    masks = make_masks()
    ident = np.tile(np.eye(64, dtype=np.float32), (2, 1))
    in_maps = []
    for c in range(N_CORES):
        h0 = c * HPC
        wq2 = round_fp32r(Wq[h0:h0 + HPC].transpose(1, 0, 2).reshape(D, 64 * HPC))
        wk2 = round_fp32r(Wk[h0:h0 + HPC].transpose(1, 0, 2).reshape(D, 64 * HPC))
        wv2 = round_fp32r(Wv[h0:h0 + HPC].transpose(1, 0, 2).reshape(D, 64 * HPC))
        woT = round_fp32r(np.ascontiguousarray(
            Wo[:, h0 * 64:(h0 + HPC) * 64].T))
        in_maps.append({
            "xT": xTr, "wq": wq2, "wk": wk2, "wv": wv2, "woT": woT,
            "masks": masks, "ident": ident,
        })
    return in_maps


def kernel(x, Wq, Wk, Wv, Wo, bo):
    from concourse.bass_utils import run_bass_kernel_spmd

    x = np.asarray(x, np.float32)
    in_maps = make_in_maps(x, np.asarray(Wq, np.float32),
                           np.asarray(Wk, np.float32),
                           np.asarray(Wv, np.float32),
                           np.asarray(Wo, np.float32))
    nc = _get_nc()
    res = run_bass_kernel_spmd(nc, in_maps, core_ids=list(range(N_CORES)))
    out = res.results[0]["y"].astype(np.float64)
    for c in range(1, N_CORES):
        out += res.results[c]["y"]
    out += np.asarray(bo, np.float64)
    return out.astype(np.float32)
